# Initial kernel scaffold
#
"""Trainium2 Bass kernel for nn_DownBlock (GNN message-passing down block).

Strategy: data-parallel over voxels across 8 NeuronCores.
 - Activated gather tables (h0 = bn_silu(x), t2 = bn_silu(h_affine), t3 = h_final)
   are materialized in DRAM per-core via AllGather; missing neighbors (-1) are
   remapped on the host to a zeroed dummy row (silu(0) == 0), so no masking is
   needed on device.
 - Gathers use multi-index indirect DMA (one instruction per supertile of 512
   voxels x 27 offsets = 13824 rows).
 - Compute runs feature-major (out_T = [128 features, voxels]): gathered tiles
   are transposed on the TensorEngine into PSUM, evacuated to SBUF in bf16 and
   fed as the moving operand of bf16 matmuls; batch-norm affine+SiLU are fused
   into one ScalarEngine activation per tile (per-partition scale/bias).
 - BN statistics are partial per-shard sums + a small AllReduce.
"""

import os
import sys
import tempfile

sys.path.insert(0, "/opt/trn_rl_repo")

import numpy as np

# ---------------------------------------------------------------------------
# Problem constants (hardcoded; kernel.py must be self-contained)
# ---------------------------------------------------------------------------
N = 200000          # voxels
NI = 64             # input features
NF = 128            # hidden features
NE = 256            # time-embedding features
B = 16              # batch entries
K = 27              # conv kernel offsets
KD = 8              # down-conv kernel offsets
ND = N // 8         # down-sampled voxels (25000)
EPS = 1e-5
CORES = 8

SHARD = N // CORES               # 25000 voxels per core
ST = 512                         # supertile = 512 voxels
SHARD_PAD = 49 * ST              # 25088 (padded shard)
NSUP = SHARD_PAD // ST           # 49 supertiles
DSHARD = ND // CORES             # 3125 down rows per core
DTILE = 128
NDT = 25                         # 25 tiles of 128 (3200 padded)
DSHARD_PAD = NDT * DTILE         # 3200
TROWS = N + 64                   # table rows incl. dummy at N
DUMMY = N                        # dummy row id (zeroed)

_COMPILED = {}


# ---------------------------------------------------------------------------
# Bass program
# ---------------------------------------------------------------------------
def build_program():
    import concourse.bacc as bacc
    import concourse.bass as bass
    import concourse.mybir as mybir
    import concourse.tile as tile

    f32 = mybir.dt.float32
    bf16 = mybir.dt.bfloat16
    i32 = mybir.dt.int32
    AF = mybir.ActivationFunctionType
    ALU = mybir.AluOpType

    nc = bacc.Bacc("TRN2", target_bir_lowering=False, debug=False)
    nc.num_devices = CORES

    # ---------------- inputs ----------------
    xT_f = nc.dram_tensor("xT_f", [NI, SHARD_PAD], f32, kind="ExternalInput")
    xT_b = nc.dram_tensor("xT_b", [NI, SHARD_PAD], bf16, kind="ExternalInput")
    idx1 = nc.dram_tensor("idx1", [128, NSUP * K * 4], i32, kind="ExternalInput")
    idxd = nc.dram_tensor("idxd", [128, NDT * KD], i32, kind="ExternalInput")
    b1hotT = nc.dram_tensor("b1hotT", [B, SHARD_PAD], bf16, kind="ExternalInput")
    tT = nc.dram_tensor("tT", [NE, B], f32, kind="ExternalInput")
    Wt_in = nc.dram_tensor("Wt", [NE, 2 * NF], f32, kind="ExternalInput")
    bt_rep = nc.dram_tensor("bt_rep", [B, 2 * NF], f32, kind="ExternalInput")
    W1s = nc.dram_tensor("W1s", [14, 128, NF], bf16, kind="ExternalInput")
    W2_in = nc.dram_tensor("W2", [K, NF, NF], bf16, kind="ExternalInput")
    Wd_in = nc.dram_tensor("Wd", [KD, NF, NF], bf16, kind="ExternalInput")
    Wid_in = nc.dram_tensor("Wid", [NI, NF], bf16, kind="ExternalInput")
    # per-feature vectors (as [P,1] columns)
    g1c = nc.dram_tensor("g1c", [NI, 1], f32, kind="ExternalInput")
    be1c = nc.dram_tensor("be1c", [NI, 1], f32, kind="ExternalInput")
    b1c = nc.dram_tensor("b1c", [NF, 1], f32, kind="ExternalInput")
    g2c = nc.dram_tensor("g2c", [NF, 1], f32, kind="ExternalInput")
    be2c = nc.dram_tensor("be2c", [NF, 1], f32, kind="ExternalInput")
    biasfc = nc.dram_tensor("biasfc", [NF, 1], f32, kind="ExternalInput")  # b2+bid
    idn_b = nc.dram_tensor("idn_b", [128, 128], bf16, kind="ExternalInput")
    idn_f = nc.dram_tensor("idn_f", [128, 128], f32, kind="ExternalInput")
    ones_b = nc.dram_tensor("ones_b", [1, ST], bf16, kind="ExternalInput")

    out_t = nc.dram_tensor("out", [DSHARD_PAD, NF], f32, kind="ExternalOutput")

    # ---------------- internal DRAM ----------------
    h0_bounce = nc.dram_tensor("h0_bounce", [SHARD, NI], bf16)
    h0_table = nc.dram_tensor("h0_table", [TROWS, NI], bf16, addr_space="Shared")
    t2_bounce = nc.dram_tensor("t2_bounce", [SHARD, NF], bf16)
    t2_table = nc.dram_tensor("t2_table", [TROWS, NF], bf16, addr_space="Shared")
    t3_bounce = nc.dram_tensor("t3_bounce", [SHARD, NF], bf16)
    t3_table = nc.dram_tensor("t3_table", [TROWS, NF], bf16, addr_space="Shared")
    st1_in = nc.dram_tensor("st1_in", [NI, 2], f32)
    st1_out = nc.dram_tensor("st1_out", [NI, 2], f32, addr_space="Shared")
    st2_in = nc.dram_tensor("st2_in", [NF, 2], f32)
    st2_out = nc.dram_tensor("st2_out", [NF, 2], f32, addr_space="Shared")

    RG = [list(range(CORES))]

    with tile.TileContext(nc) as tc:
        with (
            tc.tile_pool(name="persist", bufs=1) as pp,
            tc.tile_pool(name="sbuf", bufs=3) as sp,
            tc.tile_pool(name="gather", bufs=2) as gp,
            tc.tile_pool(name="rhs", bufs=4) as rp,
            tc.tile_pool(name="psum_st", bufs=3, space="PSUM") as pst,
            tc.tile_pool(name="psum_acc", bufs=2, space="PSUM") as pac,
            tc.tile_pool(name="psum_sm", bufs=2, space="PSUM") as psm,
        ):
            # ---------------- persistent tiles ----------------
            xTf_sb = pp.tile([NI, SHARD_PAD], f32, tag="xTf")
            idx_sb = pp.tile([128, NSUP * K * 4], i32, tag="idx")
            b1hot_sb = pp.tile([B, SHARD_PAD], bf16, tag="b1hot")
            idn_b_sb = pp.tile([128, 128], bf16, tag="idnb")
            idn_f_sb = pp.tile([128, 128], f32, tag="idnf")
            ones_sb = pp.tile([1, ST], bf16, tag="ones")
            W1_sb = pp.tile([128, 14 * NF], bf16, tag="w1")
            W2_sb = pp.tile([NF, K * NF], bf16, tag="w2")
            Wd_sb = pp.tile([NF, KD * NF], bf16, tag="wd")
            Wid_sb = pp.tile([NI, NF], bf16, tag="wid")
            vecs = pp.tile([128, 8], f32, tag="vecs")  # g1|be1|b1|g2|be2|biasf|sc|bi
            tp_bf = pp.tile([B, 2 * NF], bf16, tag="tpbf")
            haff = pp.tile([NF, SHARD_PAD], f32, tag="haff")
            stat_cols = pp.tile([128, 4 * NSUP], f32, tag="statc")  # s1|q1|s2|q2
            zero64 = pp.tile([64, NF], bf16, tag="zero64")

            nc.sync.dma_start(out=xTf_sb[:], in_=xT_f[:])
            nc.sync.dma_start(out=idx_sb[:], in_=idx1[:])
            nc.sync.dma_start(out=b1hot_sb[:], in_=b1hotT[:])
            nc.sync.dma_start(out=idn_b_sb[:], in_=idn_b[:])
            nc.sync.dma_start(out=idn_f_sb[:], in_=idn_f[:])
            nc.sync.dma_start(out=ones_sb[:], in_=ones_b[:])
            nc.sync.dma_start(
                out=W1_sb[:], in_=W1s[:].rearrange("k p n -> p (k n)")
            )
            nc.sync.dma_start(
                out=W2_sb[:], in_=W2_in[:].rearrange("k p n -> p (k n)")
            )
            nc.sync.dma_start(
                out=Wd_sb[:], in_=Wd_in[:].rearrange("k p n -> p (k n)")
            )
            nc.sync.dma_start(out=Wid_sb[:], in_=Wid_in[:])
            nc.sync.dma_start(out=vecs[:NI, 0:1], in_=g1c[:])
            nc.sync.dma_start(out=vecs[:NI, 1:2], in_=be1c[:])
            nc.sync.dma_start(out=vecs[:NF, 2:3], in_=b1c[:])
            nc.sync.dma_start(out=vecs[:NF, 3:4], in_=g2c[:])
            nc.sync.dma_start(out=vecs[:NF, 4:5], in_=be2c[:])
            nc.sync.dma_start(out=vecs[:NF, 5:6], in_=biasfc[:])
            nc.vector.memset(zero64[:], 0.0)

            # zero dummy rows of all three tables
            nc.sync.dma_start(out=h0_table[N : N + 64, :NI], in_=zero64[:, :NI])
            nc.sync.dma_start(out=t2_table[N : N + 64, :], in_=zero64[:])
            nc.sync.dma_start(out=t3_table[N : N + 64, :], in_=zero64[:])

            # ---------------- time-embedding projection ----------------
            tT_sb = sp.tile([NE, B], f32, tag="tT")
            wt_sb = sp.tile([NE, 2 * NF], f32, tag="wt")
            btr_sb = sp.tile([B, 2 * NF], f32, tag="btr")
            nc.sync.dma_start(out=tT_sb[:], in_=tT[:])
            nc.sync.dma_start(out=wt_sb[:], in_=Wt_in[:])
            nc.sync.dma_start(out=btr_sb[:], in_=bt_rep[:])
            silu_t = sp.tile([NE, B], f32, tag="silut")
            nc.scalar.activation(silu_t[:], tT_sb[:], AF.Silu)
            tp_ps = psm.tile([B, 2 * NF], f32, space="PSUM")
            nc.tensor.matmul(
                tp_ps[:], lhsT=silu_t[:128, :], rhs=wt_sb[:128, :],
                start=True, stop=False,
            )
            nc.tensor.matmul(
                tp_ps[:], lhsT=silu_t[128:, :], rhs=wt_sb[128:, :],
                start=False, stop=True,
            )
            tp_f = sp.tile([B, 2 * NF], f32, tag="tpf")
            nc.vector.tensor_add(tp_f[:], tp_ps[:], btr_sb[:])
            nc.scalar.copy(tp_bf[:], tp_f[:])  # cast to bf16

            # ---------------- phase 0: BN1 stats over xT ----------------
            sq_t = sp.tile([NI, ST], f32, tag="sq")
            for s in range(NSUP):
                xsl = xTf_sb[:, s * ST : (s + 1) * ST]
                nc.vector.reduce_sum(
                    stat_cols[:NI, s : s + 1], xsl, axis=mybir.AxisListType.X
                )
                nc.scalar.square(sq_t[:], xsl)
                nc.vector.reduce_sum(
                    stat_cols[:NI, NSUP + s : NSUP + s + 1], sq_t[:],
                    axis=mybir.AxisListType.X,
                )
            st1_sb = sp.tile([NI, 2], f32, tag="st1")
            nc.vector.reduce_sum(
                st1_sb[:, 0:1], stat_cols[:NI, 0:NSUP], axis=mybir.AxisListType.X
            )
            nc.vector.reduce_sum(
                st1_sb[:, 1:2], stat_cols[:NI, NSUP : 2 * NSUP],
                axis=mybir.AxisListType.X,
            )
            nc.sync.dma_start(out=st1_in[:], in_=st1_sb[:])
            nc.gpsimd.collective_compute(
                "AllReduce", ALU.add, replica_groups=RG,
                ins=[st1_in.ap().opt()], outs=[st1_out.ap().opt()],
            )
            st1g = sp.tile([NI, 2], f32, tag="st1g")
            nc.sync.dma_start(out=st1g[:], in_=st1_out[:])
            # mean = s/N ; var = q/N - mean^2 ; istd = 1/sqrt(var+eps)
            # scale = g1*istd ; bias = be1 - mean*scale
            mtmp = sp.tile([NI, 4], f32, tag="mtmp")
            nc.scalar.mul(mtmp[:, 0:1], st1g[:, 0:1], 1.0 / N)  # mean
            nc.scalar.mul(mtmp[:, 1:2], st1g[:, 1:2], 1.0 / N)  # E[x^2]
            nc.scalar.square(mtmp[:, 2:3], mtmp[:, 0:1])        # mean^2
            nc.vector.tensor_sub(mtmp[:, 3:4], mtmp[:, 1:2], mtmp[:, 2:3])  # var
            nc.vector.tensor_scalar_add(mtmp[:, 3:4], mtmp[:, 3:4], EPS)
            nc.scalar.sqrt(mtmp[:, 2:3], mtmp[:, 3:4])          # std
            nc.vector.reciprocal(mtmp[:, 3:4], mtmp[:, 2:3])    # istd
            nc.vector.tensor_mul(vecs[:NI, 6:7], vecs[:NI, 0:1], mtmp[:, 3:4])
            nc.vector.tensor_mul(mtmp[:, 2:3], mtmp[:, 0:1], vecs[:NI, 6:7])
            nc.vector.tensor_sub(vecs[:NI, 7:8], vecs[:NI, 1:2], mtmp[:, 2:3])

            # ---------------- phase 1: h0 table ----------------
            for s in range(NSUP):
                h0t = sp.tile([NI, ST], bf16, tag="h0t")
                nc.scalar.activation(
                    h0t[:], xTf_sb[:, s * ST : (s + 1) * ST], AF.Silu,
                    bias=vecs[:NI, 7:8], scale=vecs[:NI, 6:7],
                )
                h0r = rp.tile([128, 4 * NI], bf16, tag="h0r")
                for u in range(4):
                    tps = pst.tile([128, NI], bf16, space="PSUM", tag="h0ps")
                    nc.tensor.transpose(
                        tps[:], h0t[:, u * 128 : (u + 1) * 128],
                        idn_b_sb[:NI, :NI],
                    )
                    nc.scalar.copy(h0r[:, u * NI : (u + 1) * NI], tps[:])
                lim = min(ST, SHARD - s * ST)
                nfull = lim // 128
                if nfull > 0:
                    nc.sync.dma_start(
                        out=h0_bounce[s * ST : s * ST + nfull * 128, :].rearrange(
                            "(u p) e -> p u e", p=128
                        ),
                        in_=h0r[:, : nfull * NI].rearrange(
                            "p (u e) -> p u e", e=NI
                        ),
                    )
                rem = lim - nfull * 128
                if rem > 0:
                    nc.sync.dma_start(
                        out=h0_bounce[s * ST + nfull * 128 : s * ST + lim, :],
                        in_=h0r[:rem, nfull * NI : (nfull + 1) * NI],
                    )
            nc.gpsimd.collective_compute(
                "AllGather", ALU.bypass, replica_groups=RG,
                ins=[h0_bounce.ap().opt()], outs=[h0_table[:N, :].opt()],
            )

            # ---------------- phase 2: conv1 + time affine ----------------
            KP = 14  # 13 full k-pairs + 1 (k=26 duplicated, zero weights)
            for s in range(NSUP):
                g1t = gp.tile([128, K * 4 * NI], bf16, tag="g1t")
                nc.gpsimd.indirect_dma_start(
                    out=g1t[:].rearrange("p (m e) -> p m e", e=NI),
                    out_offset=None,
                    in_=h0_table[:],
                    in_offset=bass.IndirectOffsetOnAxis(
                        ap=idx_sb[:, s * K * 4 : (s + 1) * K * 4], axis=0
                    ),
                )
                acc = pac.tile([NF, ST], f32, space="PSUM", tag="acc1")
                for kp in range(KP):
                    ka, kb = 2 * kp, min(2 * kp + 1, K - 1)
                    stg = pst.tile([128, ST], bf16, space="PSUM", tag="stg1")
                    for u in range(4):
                        ca = (ka * 4 + u) * NI
                        cb = (kb * 4 + u) * NI
                        nc.tensor.transpose(
                            stg[:NI, u * 128 : (u + 1) * 128],
                            g1t[:, ca : ca + NI], idn_b_sb[:NI, :NI],
                        )
                        nc.tensor.transpose(
                            stg[NI:, u * 128 : (u + 1) * 128],
                            g1t[:, cb : cb + NI], idn_b_sb[:NI, :NI],
                        )
                    rhs = rp.tile([128, ST], bf16, tag="rhs1")
                    eng = nc.vector if (kp % 2 == 0) else nc.scalar
                    eng.tensor_copy(rhs[:], stg[:]) if kp % 2 == 0 else eng.copy(
                        rhs[:], stg[:]
                    )
                    nc.tensor.matmul(
                        acc[:], lhsT=W1_sb[:, kp * NF : (kp + 1) * NF],
                        rhs=rhs[:], start=(kp == 0), stop=(kp == KP - 1),
                    )
                # C = conv + b1 (per-partition bias)
                ct = sp.tile([NF, ST], f32, tag="ct")
                nc.scalar.activation(
                    ct[:], acc[:], AF.Copy_w_bias, bias=vecs[:NF, 2:3]
                )
                # scale/shift via one-hot matmuls
                sps = psm.tile([NF, ST], f32, space="PSUM", tag="sps")
                nc.tensor.matmul(
                    sps[:], lhsT=tp_bf[:, :NF],
                    rhs=b1hot_sb[:, s * ST : (s + 1) * ST],
                    start=True, stop=False,
                )
                nc.tensor.matmul(
                    sps[:], lhsT=ones_sb[:, :NF].bitcast(bf16),
                    rhs=ones_sb[:], start=False, stop=True,
                )  # += 1
                shp = psm.tile([NF, ST], f32, space="PSUM", tag="shp")
                nc.tensor.matmul(
                    shp[:], lhsT=tp_bf[:, NF:],
                    rhs=b1hot_sb[:, s * ST : (s + 1) * ST],
                    start=True, stop=True,
                )
                hsl = haff[:, s * ST : (s + 1) * ST]
                tmp = sp.tile([NF, ST], f32, tag="tmp1")
                nc.vector.tensor_mul(tmp[:], sps[:], ct[:])
                nc.vector.tensor_add(hsl, tmp[:], shp[:])
            # zero padded columns before stats
            nc.vector.memset(haff[:, SHARD:SHARD_PAD], 0.0)
            # BN2 stats
            for s in range(NSUP):
                hsl = haff[:, s * ST : (s + 1) * ST]
                nc.vector.reduce_sum(
                    stat_cols[:NF, 2 * NSUP + s : 2 * NSUP + s + 1], hsl,
                    axis=mybir.AxisListType.X,
                )
                sq2 = sp.tile([NF, ST], f32, tag="sq")
                nc.scalar.square(sq2[:], hsl)
                nc.vector.reduce_sum(
                    stat_cols[:NF, 3 * NSUP + s : 3 * NSUP + s + 1], sq2[:],
                    axis=mybir.AxisListType.X,
                )
            st2_sb = sp.tile([NF, 2], f32, tag="st2")
            nc.vector.reduce_sum(
                st2_sb[:, 0:1], stat_cols[:NF, 2 * NSUP : 3 * NSUP],
                axis=mybir.AxisListType.X,
            )
            nc.vector.reduce_sum(
                st2_sb[:, 1:2], stat_cols[:NF, 3 * NSUP : 4 * NSUP],
                axis=mybir.AxisListType.X,
            )
            nc.sync.dma_start(out=st2_in[:], in_=st2_sb[:])
            nc.gpsimd.collective_compute(
                "AllReduce", ALU.add, replica_groups=RG,
                ins=[st2_in.ap().opt()], outs=[st2_out.ap().opt()],
            )
            st2g = sp.tile([NF, 2], f32, tag="st2g")
            nc.sync.dma_start(out=st2g[:], in_=st2_out[:])
            m2 = sp.tile([NF, 4], f32, tag="m2")
            nc.scalar.mul(m2[:, 0:1], st2g[:, 0:1], 1.0 / N)
            nc.scalar.mul(m2[:, 1:2], st2g[:, 1:2], 1.0 / N)
            nc.scalar.square(m2[:, 2:3], m2[:, 0:1])
            nc.vector.tensor_sub(m2[:, 3:4], m2[:, 1:2], m2[:, 2:3])
            nc.vector.tensor_scalar_add(m2[:, 3:4], m2[:, 3:4], EPS)
            nc.scalar.sqrt(m2[:, 2:3], m2[:, 3:4])
            nc.vector.reciprocal(m2[:, 3:4], m2[:, 2:3])
            sc2 = sp.tile([NF, 2], f32, tag="sc2")
            nc.vector.tensor_mul(sc2[:, 0:1], vecs[:NF, 3:4], m2[:, 3:4])
            nc.vector.tensor_mul(m2[:, 2:3], m2[:, 0:1], sc2[:, 0:1])
            nc.vector.tensor_sub(sc2[:, 1:2], vecs[:NF, 4:5], m2[:, 2:3])

            # ---------------- phase 3: t2 table ----------------
            for s in range(NSUP):
                t2t = sp.tile([NF, ST], bf16, tag="t2t")
                nc.scalar.activation(
                    t2t[:], haff[:, s * ST : (s + 1) * ST], AF.Silu,
                    bias=sc2[:, 1:2], scale=sc2[:, 0:1],
                )
                t2r = rp.tile([128, 4 * NF], bf16, tag="t2r")
                for u in range(4):
                    tps = pst.tile([128, NF], bf16, space="PSUM", tag="t2ps")
                    nc.tensor.transpose(
                        tps[:], t2t[:, u * 128 : (u + 1) * 128], idn_b_sb[:]
                    )
                    nc.scalar.copy(t2r[:, u * NF : (u + 1) * NF], tps[:])
                lim = min(ST, SHARD - s * ST)
                nfull = lim // 128
                if nfull > 0:
                    nc.sync.dma_start(
                        out=t2_bounce[s * ST : s * ST + nfull * 128, :].rearrange(
                            "(u p) e -> p u e", p=128
                        ),
                        in_=t2r[:, : nfull * NF].rearrange(
                            "p (u e) -> p u e", e=NF
                        ),
                    )
                rem = lim - nfull * 128
                if rem > 0:
                    nc.sync.dma_start(
                        out=t2_bounce[s * ST + nfull * 128 : s * ST + lim, :],
                        in_=t2r[:rem, nfull * NF : (nfull + 1) * NF],
                    )
            nc.gpsimd.collective_compute(
                "AllGather", ALU.bypass, replica_groups=RG,
                ins=[t2_bounce.ap().opt()], outs=[t2_table[:N, :].opt()],
            )

            # ---------------- phase 4: conv2 + idconv -> t3 table ----------------
            for s in range(NSUP):
                g2t = gp.tile([128, K * 4 * NF], bf16, tag="g2t")
                nc.gpsimd.indirect_dma_start(
                    out=g2t[:].rearrange("p (m e) -> p m e", e=NF),
                    out_offset=None,
                    in_=t2_table[:],
                    in_offset=bass.IndirectOffsetOnAxis(
                        ap=idx_sb[:, s * K * 4 : (s + 1) * K * 4], axis=0
                    ),
                )
                acc2 = pac.tile([NF, ST], f32, space="PSUM", tag="acc2")
                xbt = sp.tile([NI, ST], bf16, tag="xbt")
                nc.sync.dma_start(
                    out=xbt[:], in_=xT_b[:, s * ST : (s + 1) * ST]
                )
                nc.tensor.matmul(
                    acc2[:], lhsT=Wid_sb[:], rhs=xbt[:],
                    start=True, stop=False,
                )
                for k in range(K):
                    stg = pst.tile([128, ST], bf16, space="PSUM", tag="stg2")
                    for u in range(4):
                        cc = (k * 4 + u) * NF
                        nc.tensor.transpose(
                            stg[:, u * 128 : (u + 1) * 128],
                            g2t[:, cc : cc + NF], idn_b_sb[:],
                        )
                    rhs = rp.tile([128, ST], bf16, tag="rhs2")
                    if k % 2 == 0:
                        nc.vector.tensor_copy(rhs[:], stg[:])
                    else:
                        nc.scalar.copy(rhs[:], stg[:])
                    nc.tensor.matmul(
                        acc2[:], lhsT=W2_sb[:, k * NF : (k + 1) * NF],
                        rhs=rhs[:], start=False, stop=(k == K - 1),
                    )
                # h_final = acc2 + (b2+bid), cast bf16
                t3t = sp.tile([NF, ST], bf16, tag="t3t")
                nc.scalar.activation(
                    t3t[:], acc2[:], AF.Copy_w_bias, bias=vecs[:NF, 5:6]
                )
                t3r = rp.tile([128, 4 * NF], bf16, tag="t3r")
                for u in range(4):
                    tps = pst.tile([128, NF], bf16, space="PSUM", tag="t3ps")
                    nc.tensor.transpose(
                        tps[:], t3t[:, u * 128 : (u + 1) * 128], idn_b_sb[:]
                    )
                    nc.scalar.copy(t3r[:, u * NF : (u + 1) * NF], tps[:])
                lim = min(ST, SHARD - s * ST)
                nfull = lim // 128
                if nfull > 0:
                    nc.sync.dma_start(
                        out=t3_bounce[s * ST : s * ST + nfull * 128, :].rearrange(
                            "(u p) e -> p u e", p=128
                        ),
                        in_=t3r[:, : nfull * NF].rearrange(
                            "p (u e) -> p u e", e=NF
                        ),
                    )
                rem = lim - nfull * 128
                if rem > 0:
                    nc.sync.dma_start(
                        out=t3_bounce[s * ST + nfull * 128 : s * ST + lim, :],
                        in_=t3r[:rem, nfull * NF : (nfull + 1) * NF],
                    )
            nc.gpsimd.collective_compute(
                "AllGather", ALU.bypass, replica_groups=RG,
                ins=[t3_bounce.ap().opt()], outs=[t3_table[:N, :].opt()],
            )

            # ---------------- phase 5: down conv ----------------
            idxd_sb = pp.tile([128, NDT * KD], i32, tag="idxd")
            nc.sync.dma_start(out=idxd_sb[:], in_=idxd[:])
            for td in range(NDT):
                gdt = gp.tile([128, KD * NF], bf16, tag="gdt")
                nc.gpsimd.indirect_dma_start(
                    out=gdt[:].rearrange("p (m e) -> p m e", e=NF),
                    out_offset=None,
                    in_=t3_table[:],
                    in_offset=bass.IndirectOffsetOnAxis(
                        ap=idxd_sb[:, td * KD : (td + 1) * KD], axis=0
                    ),
                )
                accd = pac.tile([NF, DTILE], f32, space="PSUM", tag="accd")
                for kd in range(KD):
                    stg = pst.tile([128, DTILE], bf16, space="PSUM", tag="stgd")
                    nc.tensor.transpose(
                        stg[:], gdt[:, kd * NF : (kd + 1) * NF], idn_b_sb[:]
                    )
                    rhs = rp.tile([128, DTILE], bf16, tag="rhsd")
                    if kd % 2 == 0:
                        nc.vector.tensor_copy(rhs[:], stg[:])
                    else:
                        nc.scalar.copy(rhs[:], stg[:])
                    nc.tensor.matmul(
                        accd[:], lhsT=Wd_sb[:, kd * NF : (kd + 1) * NF],
                        rhs=rhs[:], start=(kd == 0), stop=(kd == KD - 1),
                    )
                od_f = sp.tile([NF, DTILE], f32, tag="odf")
                nc.vector.tensor_copy(od_f[:], accd[:])
                ops = pst.tile([128, NF], f32, space="PSUM", tag="ops")
                nc.tensor.transpose(ops[:], od_f[:], idn_f_sb[:])
                od_r = sp.tile([128, NF], f32, tag="odr")
                nc.scalar.copy(od_r[:], ops[:])
                nc.sync.dma_start(
                    out=out_t[td * DTILE : (td + 1) * DTILE, :], in_=od_r[:]
                )

    nc.compile()
    return nc


# ---------------------------------------------------------------------------
# Host-side sharding / input prep
# ---------------------------------------------------------------------------
def prep_inputs(x, t, b_idx, nbr, nbr_down, g1, be1, W1, b1, Wt, bt,
                g2, be2, W2, b2, Wid, bid, Wd):
    import ml_dtypes

    bf16 = ml_dtypes.bfloat16
    f32 = np.float32

    nbr_fixed = np.where(nbr < 0, DUMMY, nbr).astype(np.int32)  # [K, N]

    # pair-stacked W1 (+ zero pad for odd k)
    W1s = np.zeros((14, 128, NF), dtype=bf16)
    for kp in range(13):
        W1s[kp, :NI] = W1[2 * kp].astype(bf16)
        W1s[kp, NI:] = W1[2 * kp + 1].astype(bf16)
    W1s[13, :NI] = W1[26].astype(bf16)  # second half stays zero

    bt_rep = np.broadcast_to(bt.astype(f32), (B, 2 * NF)).copy()
    idn = np.eye(128)

    common = dict(
        tT=np.ascontiguousarray(t.astype(f32).T),
        Wt=np.ascontiguousarray(Wt.astype(f32)),
        bt_rep=bt_rep,
        W1s=W1s,
        W2=np.ascontiguousarray(W2.astype(bf16)),
        Wd=np.ascontiguousarray(Wd.astype(bf16)),
        Wid=np.ascontiguousarray(Wid.astype(bf16)),
        g1c=np.ascontiguousarray(g1.astype(f32).reshape(NI, 1)),
        be1c=np.ascontiguousarray(be1.astype(f32).reshape(NI, 1)),
        b1c=np.ascontiguousarray(b1.astype(f32).reshape(NF, 1)),
        g2c=np.ascontiguousarray(g2.astype(f32).reshape(NF, 1)),
        be2c=np.ascontiguousarray(be2.astype(f32).reshape(NF, 1)),
        biasfc=np.ascontiguousarray((b2 + bid).astype(f32).reshape(NF, 1)),
        idn_b=idn.astype(bf16),
        idn_f=idn.astype(f32),
        ones_b=np.ones((1, ST), dtype=bf16),
    )

    in_maps = []
    for c in range(CORES):
        lo, hi = c * SHARD, (c + 1) * SHARD
        xs = x[lo:hi].astype(f32)                      # [SHARD, NI]
        xT = np.zeros((NI, SHARD_PAD), dtype=f32)
        xT[:, :SHARD] = xs.T

        # conv neighbor indices: [128, NSUP*K*4], col m = s*K*4 + k*4 + u
        nb = nbr_fixed[:, lo:hi]                       # [K, SHARD]
        nb_pad = np.full((K, SHARD_PAD), DUMMY, dtype=np.int32)
        nb_pad[:, :SHARD] = nb
        # [K, NSUP, 4, 128] -> [128, NSUP, K, 4]
        nbv = nb_pad.reshape(K, NSUP, 4, 128).transpose(3, 1, 0, 2)
        idx1 = np.ascontiguousarray(nbv.reshape(128, NSUP * K * 4))

        # one-hot of b_idx, transposed: [B, SHARD_PAD]
        bh = np.zeros((B, SHARD_PAD), dtype=bf16)
        bh[b_idx[lo:hi].astype(np.int64), np.arange(SHARD)] = 1

        # down-conv indices: [128, NDT*KD], col m = td*KD + kd
        dlo = c * DSHARD
        nd_pad = np.full((KD, DSHARD_PAD), DUMMY, dtype=np.int32)
        nd_pad[:, :DSHARD] = nbr_down[:, dlo : dlo + DSHARD]
        ndv = nd_pad.reshape(KD, NDT, 128).transpose(2, 1, 0)
        idxd = np.ascontiguousarray(ndv.reshape(128, NDT * KD))

        in_maps.append(
            dict(
                common,
                xT_f=xT,
                xT_b=xT.astype(bf16),
                idx1=idx1,
                idxd=idxd,
                b1hotT=bh,
            )
        )
    return in_maps


def kernel(**inputs):
    from concourse.bass_utils import run_bass_kernel_spmd

    if "nc" not in _COMPILED:
        _COMPILED["nc"] = build_program()
    nc = _COMPILED["nc"]

    inputs = {k: np.asarray(v) for k, v in inputs.items()}
    in_maps = prep_inputs(**inputs)
    res = run_bass_kernel_spmd(nc, in_maps, core_ids=list(range(CORES)))
    out = np.concatenate(
        [res.results[c]["out"][:DSHARD] for c in range(CORES)], axis=0
    )
    return out.astype(np.float32)


if __name__ == "__main__":
    nc = build_program()
    print("program built OK")


# revision 9
# speedup vs baseline: 25.4897x; 25.4897x over previous
"""Trainium2 Bass kernel for nn_DownBlock (GNN message-passing down block).

Data-parallel over voxels across 8 NeuronCores.
 - Activated gather tables (h0 = bn_silu(x), t2 = bn_silu(h_affine), t3 = h_final)
   are materialized in DRAM per-core via AllGather; missing neighbors (-1) are
   remapped on the host to a zeroed dummy row (silu(0) == 0), so no masking is
   needed on device.
 - Gathers use multi-index indirect DMA (one instruction per supertile of 512
   voxels x 27 offsets).
 - Compute runs feature-major (out_T = [features, voxels]): gathered tiles are
   transposed on the TensorEngine into PSUM, evacuated to SBUF in bf16 and fed
   as the moving operand of bf16 matmuls; batch-norm affine+SiLU fuse into one
   ScalarEngine activation per tile (per-partition scale/bias).
 - BN statistics are partial per-shard sums + a small AllReduce.
"""

import sys

sys.path.insert(0, "/opt/trn_rl_repo")

import numpy as np

# ---------------------------------------------------------------------------
# Problem constants (hardcoded; kernel.py must be self-contained)
# ---------------------------------------------------------------------------
N = 200000          # voxels
NI = 64             # input features
NF = 128            # hidden features
NE = 256            # time-embedding features
B = 16              # batch entries
K = 27              # conv kernel offsets
KD = 8              # down-conv kernel offsets
ND = N // 8         # down-sampled voxels (25000)
EPS = 1e-5
CORES = 8

SHARD = N // CORES               # 25000 voxels per core
ST = 512                         # supertile = 512 voxels
NSUP = 49                        # supertiles per shard
SHARD_PAD = NSUP * ST            # 25088 (padded shard)
DSHARD = ND // CORES             # 3125 down rows per core
DTILE = 128
NDT = 25                         # 25 tiles of 128 (3200 padded)
DSHARD_PAD = NDT * DTILE         # 3200
TROWS = N + 64                   # table rows incl. dummy at N
DUMMY = N                        # dummy row id (zeroed)

_COMPILED = {}


# ---------------------------------------------------------------------------
# Bass program
# ---------------------------------------------------------------------------
def build_program():
    import concourse.bacc as bacc
    import concourse.bass as bass
    import concourse.mybir as mybir
    import concourse.tile as tile

    f32 = mybir.dt.float32
    bf16 = mybir.dt.bfloat16
    i32 = mybir.dt.int32
    AF = mybir.ActivationFunctionType
    ALU = mybir.AluOpType
    AX = mybir.AxisListType.X

    nc = bacc.Bacc("TRN2", target_bir_lowering=False, debug=False,
                   num_devices=CORES)

    # ---------------- inputs ----------------
    xT_f = nc.dram_tensor("xT_f", [NI, SHARD_PAD], f32, kind="ExternalInput")
    xT_b = nc.dram_tensor("xT_b", [NI, SHARD_PAD], bf16, kind="ExternalInput")
    idx1 = nc.dram_tensor("idx1", [128, NSUP * K * 4], i32, kind="ExternalInput")
    idxd = nc.dram_tensor("idxd", [128, NDT * KD], i32, kind="ExternalInput")
    b1hotT = nc.dram_tensor("b1hotT", [B, SHARD_PAD], bf16, kind="ExternalInput")
    tT = nc.dram_tensor("tT", [128, 2 * B], f32, kind="ExternalInput")
    Wt_in = nc.dram_tensor("Wt", [128, 4 * NF], f32, kind="ExternalInput")
    bt_rep = nc.dram_tensor("bt_rep", [B, 2 * NF], f32, kind="ExternalInput")
    W1s = nc.dram_tensor("W1s", [128, 14 * NF], bf16, kind="ExternalInput")
    W2_in = nc.dram_tensor("W2", [NF, K * NF], bf16, kind="ExternalInput")
    Wd_in = nc.dram_tensor("Wd", [NF, KD * NF], bf16, kind="ExternalInput")
    Wid_in = nc.dram_tensor("Wid", [NI, NF], bf16, kind="ExternalInput")
    g1c = nc.dram_tensor("g1c", [NI, 1], f32, kind="ExternalInput")
    be1c = nc.dram_tensor("be1c", [NI, 1], f32, kind="ExternalInput")
    b1c = nc.dram_tensor("b1c", [NF, 1], f32, kind="ExternalInput")
    g2c = nc.dram_tensor("g2c", [NF, 1], f32, kind="ExternalInput")
    be2c = nc.dram_tensor("be2c", [NF, 1], f32, kind="ExternalInput")
    biasfc = nc.dram_tensor("biasfc", [NF, 1], f32, kind="ExternalInput")  # b2+bid
    idn_b = nc.dram_tensor("idn_b", [128, 128], bf16, kind="ExternalInput")
    idn_f = nc.dram_tensor("idn_f", [128, 128], f32, kind="ExternalInput")
    ones_b = nc.dram_tensor("ones_b", [1, ST], bf16, kind="ExternalInput")

    out_t = nc.dram_tensor("out", [DSHARD_PAD, NF], f32, kind="ExternalOutput")

    # ---------------- internal DRAM ----------------
    h0_bounce = nc.dram_tensor("h0_bounce", [SHARD, NI], bf16)
    h0_table = nc.dram_tensor("h0_table", [TROWS, NI], bf16, addr_space="Shared")
    t2_bounce = nc.dram_tensor("t2_bounce", [SHARD, NF], bf16)
    t2_table = nc.dram_tensor("t2_table", [TROWS, NF], bf16, addr_space="Shared")
    t3_bounce = nc.dram_tensor("t3_bounce", [SHARD, NF], bf16)
    t3_table = nc.dram_tensor("t3_table", [TROWS, NF], bf16, addr_space="Shared")
    st1_in = nc.dram_tensor("st1_in", [NI, 2], f32)
    st1_out = nc.dram_tensor("st1_out", [NI, 2], f32, addr_space="Shared")
    st2_in = nc.dram_tensor("st2_in", [NF, 2], f32)
    st2_out = nc.dram_tensor("st2_out", [NF, 2], f32, addr_space="Shared")

    RG = [list(range(CORES))]

    def store_rows(dram, s, src_tile, width):
        """DMA a [128, 4*width] row-chunk tile back to row-major DRAM."""
        lim = min(ST, SHARD - s * ST)
        nfull = lim // 128
        if nfull > 0:
            nc.sync.dma_start(
                out=dram[s * ST : s * ST + nfull * 128, :].rearrange(
                    "(u p) e -> p u e", p=128
                ),
                in_=src_tile[:, : nfull * width].rearrange(
                    "p (u e) -> p u e", e=width
                ),
            )
        rem = lim - nfull * 128
        if rem > 0:
            nc.sync.dma_start(
                out=dram[s * ST + nfull * 128 : s * ST + lim, :],
                in_=src_tile[:rem, nfull * width : (nfull + 1) * width],
            )

    with tile.TileContext(nc) as tc:
        with (
            tc.tile_pool(name="persist", bufs=1) as pp,
            tc.tile_pool(name="sbuf", bufs=3) as sp,
            tc.tile_pool(name="gather", bufs=2) as gp,
            tc.tile_pool(name="rhs", bufs=4) as rp,
            tc.tile_pool(name="psum_st", bufs=3, space="PSUM") as pst,
            tc.tile_pool(name="psum_acc", bufs=2, space="PSUM") as pac,
            tc.tile_pool(name="psum_sm", bufs=1, space="PSUM") as psm,
        ):
            # ---------------- persistent tiles ----------------
            idx_sb = pp.tile([128, NSUP * K * 4], i32, tag="idx")
            idxd_sb = pp.tile([128, NDT * KD], i32, tag="idxd")
            idn_b_sb = pp.tile([128, 128], bf16, tag="idnb")
            idn_f_sb = pp.tile([128, 128], f32, tag="idnf")
            ones_sb = pp.tile([1, ST], bf16, tag="ones")
            W1_sb = pp.tile([128, 14 * NF], bf16, tag="w1")
            W2_sb = pp.tile([NF, K * NF], bf16, tag="w2")
            Wd_sb = pp.tile([NF, KD * NF], bf16, tag="wd")
            Wid_sb = pp.tile([NI, NF], bf16, tag="wid")
            vecs = pp.tile([128, 8], f32, tag="vecs")  # g1|be1|b1|g2|be2|biasf|sc1|bi1
            tp_bf = pp.tile([B, 2 * NF], bf16, tag="tpbf")
            haff = pp.tile([NF, SHARD_PAD], bf16, tag="haff")
            stat_cols = pp.tile([128, 4 * NSUP], f32, tag="statc")  # s1|q1|s2|q2
            zero64 = pp.tile([64, NF], bf16, tag="zero64")

            nc.sync.dma_start(out=idx_sb[:], in_=idx1[:])
            nc.sync.dma_start(out=idxd_sb[:], in_=idxd[:])
            nc.sync.dma_start(out=idn_b_sb[:], in_=idn_b[:])
            nc.sync.dma_start(out=idn_f_sb[:], in_=idn_f[:])
            nc.sync.dma_start(out=ones_sb[:], in_=ones_b[:])
            nc.sync.dma_start(out=W1_sb[:], in_=W1s[:])
            nc.sync.dma_start(out=W2_sb[:], in_=W2_in[:])
            nc.sync.dma_start(out=Wd_sb[:], in_=Wd_in[:])
            nc.sync.dma_start(out=Wid_sb[:], in_=Wid_in[:])
            nc.sync.dma_start(out=vecs[:NI, 0:1], in_=g1c[:])
            nc.sync.dma_start(out=vecs[:NI, 1:2], in_=be1c[:])
            nc.sync.dma_start(out=vecs[:NF, 2:3], in_=b1c[:])
            nc.sync.dma_start(out=vecs[:NF, 3:4], in_=g2c[:])
            nc.sync.dma_start(out=vecs[:NF, 4:5], in_=be2c[:])
            nc.sync.dma_start(out=vecs[:NF, 5:6], in_=biasfc[:])
            nc.vector.memset(zero64[:], 0.0)

            # zero dummy rows of all three tables
            nc.sync.dma_start(out=h0_table[N : N + 64, :NI], in_=zero64[:, :NI])
            nc.sync.dma_start(out=t2_table[N : N + 64, :], in_=zero64[:])
            nc.sync.dma_start(out=t3_table[N : N + 64, :], in_=zero64[:])

            # ---------------- time-embedding projection ----------------
            tT_sb = sp.tile([128, 2 * B], f32, tag="tT")
            wt_sb = sp.tile([128, 4 * NF], f32, tag="wt")
            btr_sb = sp.tile([B, 2 * NF], f32, tag="btr")
            nc.sync.dma_start(out=tT_sb[:], in_=tT[:])
            nc.sync.dma_start(out=wt_sb[:], in_=Wt_in[:])
            nc.sync.dma_start(out=btr_sb[:], in_=bt_rep[:])
            silu_t = sp.tile([128, 2 * B], f32, tag="silut")
            nc.scalar.activation(silu_t[:], tT_sb[:], AF.Silu)
            tp_ps = psm.tile([B, 2 * NF], f32, tag="sps")
            nc.tensor.matmul(tp_ps[:], lhsT=silu_t[:, :B],
                             rhs=wt_sb[:, : 2 * NF], start=True, stop=False)
            nc.tensor.matmul(tp_ps[:], lhsT=silu_t[:, B:],
                             rhs=wt_sb[:, 2 * NF :], start=False, stop=True)
            tp_f = sp.tile([B, 2 * NF], f32, tag="tpf")
            nc.vector.tensor_add(tp_f[:], tp_ps[:], btr_sb[:])
            nc.scalar.copy(tp_bf[:], tp_f[:])  # cast to bf16

            # ---------------- phase 0: BN1 stats over xT ----------------
            for s in range(NSUP):
                xsl = sp.tile([NI, ST], f32, tag="xsl")
                nc.sync.dma_start(out=xsl[:], in_=xT_f[:, s * ST : (s + 1) * ST])
                nc.vector.reduce_sum(stat_cols[:NI, s : s + 1], xsl[:], axis=AX)
                sq_t = sp.tile([NI, ST], f32, tag="sq")
                nc.scalar.square(sq_t[:], xsl[:])
                nc.vector.reduce_sum(
                    stat_cols[:NI, NSUP + s : NSUP + s + 1], sq_t[:], axis=AX
                )
            st1_sb = sp.tile([NI, 2], f32, tag="st1")
            nc.vector.reduce_sum(st1_sb[:, 0:1], stat_cols[:NI, 0:NSUP], axis=AX)
            nc.vector.reduce_sum(
                st1_sb[:, 1:2], stat_cols[:NI, NSUP : 2 * NSUP], axis=AX
            )
            nc.sync.dma_start(out=st1_in[:], in_=st1_sb[:])
            nc.gpsimd.collective_compute(
                "AllReduce", ALU.add, replica_groups=RG,
                ins=[st1_in.ap().opt()], outs=[st1_out.ap().opt()],
            )
            st1g = sp.tile([NI, 2], f32, tag="st1g")
            nc.sync.dma_start(out=st1g[:], in_=st1_out[:])
            # mean = s/N ; var = q/N - mean^2 ; istd = 1/sqrt(var+eps)
            # scale = g1*istd ; bias = be1 - mean*scale
            mtmp = sp.tile([NI, 4], f32, tag="mtmp")
            nc.scalar.mul(mtmp[:, 0:1], st1g[:, 0:1], 1.0 / N)   # mean
            nc.scalar.mul(mtmp[:, 1:2], st1g[:, 1:2], 1.0 / N)   # E[x^2]
            nc.scalar.square(mtmp[:, 2:3], mtmp[:, 0:1])         # mean^2
            nc.vector.tensor_sub(mtmp[:, 3:4], mtmp[:, 1:2], mtmp[:, 2:3])
            nc.vector.tensor_scalar_add(mtmp[:, 3:4], mtmp[:, 3:4], EPS)
            nc.scalar.sqrt(mtmp[:, 2:3], mtmp[:, 3:4])           # std
            nc.vector.reciprocal(mtmp[:, 3:4], mtmp[:, 2:3])     # istd
            nc.vector.tensor_mul(vecs[:NI, 6:7], vecs[:NI, 0:1], mtmp[:, 3:4])
            nc.vector.tensor_mul(mtmp[:, 2:3], mtmp[:, 0:1], vecs[:NI, 6:7])
            nc.vector.tensor_sub(vecs[:NI, 7:8], vecs[:NI, 1:2], mtmp[:, 2:3])

            # ---------------- phase 1: h0 table ----------------
            for s in range(NSUP):
                xsl = sp.tile([NI, ST], f32, tag="xsl")
                nc.sync.dma_start(out=xsl[:], in_=xT_f[:, s * ST : (s + 1) * ST])
                h0t = sp.tile([NI, ST], bf16, tag="h0t")
                nc.scalar.activation(
                    h0t[:], xsl[:], AF.Silu,
                    bias=vecs[:NI, 7:8], scale=vecs[:NI, 6:7],
                )
                h0r = rp.tile([128, 4 * NI], bf16, tag="rT")
                for u in range(4):
                    tps = pst.tile([128, ST], bf16, tag="stg")
                    nc.tensor.transpose(
                        tps[:, :NI], h0t[:, u * 128 : (u + 1) * 128],
                        idn_b_sb[:NI, :NI],
                    )
                    nc.scalar.copy(h0r[:, u * NI : (u + 1) * NI], tps[:, :NI])
                store_rows(h0_bounce, s, h0r, NI)
            nc.gpsimd.collective_compute(
                "AllGather", ALU.bypass, replica_groups=RG,
                ins=[h0_bounce.ap().opt()], outs=[h0_table[:N, :].opt()],
            )

            # ---------------- phase 2: conv1 + time affine ----------------
            KP = 14  # 13 full k-pairs + 1 (k=26 duplicated, zero weights)
            for s in range(NSUP):
                g1t = gp.tile([128, K * 4 * NF], bf16, tag="gt")
                nc.gpsimd.indirect_dma_start(
                    out=g1t[:, : K * 4 * NI].rearrange("p (m e) -> p m e", e=NI),
                    out_offset=None,
                    in_=h0_table[:],
                    in_offset=bass.IndirectOffsetOnAxis(
                        ap=idx_sb[:, s * K * 4 : (s + 1) * K * 4], axis=0
                    ),
                )
                bh_sl = sp.tile([B, ST], bf16, tag="bhsl")
                nc.sync.dma_start(
                    out=bh_sl[:], in_=b1hotT[:, s * ST : (s + 1) * ST]
                )
                acc = pac.tile([NF, ST], f32, tag="acc")
                for kp in range(KP):
                    ka, kb = 2 * kp, min(2 * kp + 1, K - 1)
                    stg = pst.tile([128, ST], bf16, tag="stg")
                    for u in range(4):
                        ca = (ka * 4 + u) * NI
                        cb = (kb * 4 + u) * NI
                        nc.tensor.transpose(
                            stg[:NI, u * 128 : (u + 1) * 128],
                            g1t[:, ca : ca + NI], idn_b_sb[:],
                        )
                        nc.tensor.transpose(
                            stg[NI:, u * 128 : (u + 1) * 128],
                            g1t[:, cb : cb + NI], idn_b_sb[:],
                        )
                    rhs = rp.tile([128, ST], bf16, tag="rhs")
                    if kp % 2 == 0:
                        nc.vector.tensor_copy(rhs[:], stg[:])
                    else:
                        nc.scalar.copy(rhs[:], stg[:])
                    nc.tensor.matmul(
                        acc[:], lhsT=W1_sb[:, kp * NF : (kp + 1) * NF],
                        rhs=rhs[:], start=(kp == 0), stop=(kp == KP - 1),
                    )
                # C = conv + b1 (per-partition bias)
                ct = sp.tile([NF, ST], f32, tag="ct")
                nc.scalar.activation(ct[:], acc[:], AF.Identity,
                                     bias=vecs[:NF, 2:3])
                # s1 = 1 + scale[b_idx] ; sh = shift[b_idx] (one-hot matmuls)
                sps = psm.tile([NF, ST], f32, tag="sps")
                nc.tensor.matmul(sps[:], lhsT=tp_bf[:, :NF], rhs=bh_sl[:],
                                 start=True, stop=False)
                nc.tensor.matmul(sps[:], lhsT=ones_sb[:, :NF], rhs=ones_sb[:],
                                 start=False, stop=True)  # += 1
                shp = psm.tile([NF, ST], f32, tag="shp")
                nc.tensor.matmul(shp[:], lhsT=tp_bf[:, NF:], rhs=bh_sl[:],
                                 start=True, stop=True)
                tmp = sp.tile([NF, ST], f32, tag="tmp1")
                nc.vector.tensor_mul(tmp[:], sps[:], ct[:])
                nc.vector.tensor_add(haff[:, s * ST : (s + 1) * ST], tmp[:], shp[:])
            # zero padded columns before stats
            nc.vector.memset(haff[:, SHARD:SHARD_PAD], 0.0)
            # BN2 stats
            for s in range(NSUP):
                hsl = haff[:, s * ST : (s + 1) * ST]
                nc.vector.reduce_sum(
                    stat_cols[:NF, 2 * NSUP + s : 2 * NSUP + s + 1], hsl, axis=AX
                )
                sq2 = sp.tile([NF, ST], f32, tag="sq")
                nc.scalar.square(sq2[:], hsl)
                nc.vector.reduce_sum(
                    stat_cols[:NF, 3 * NSUP + s : 3 * NSUP + s + 1], sq2[:], axis=AX
                )
            st2_sb = sp.tile([NF, 2], f32, tag="st2")
            nc.vector.reduce_sum(
                st2_sb[:, 0:1], stat_cols[:NF, 2 * NSUP : 3 * NSUP], axis=AX
            )
            nc.vector.reduce_sum(
                st2_sb[:, 1:2], stat_cols[:NF, 3 * NSUP : 4 * NSUP], axis=AX
            )
            nc.sync.dma_start(out=st2_in[:], in_=st2_sb[:])
            nc.gpsimd.collective_compute(
                "AllReduce", ALU.add, replica_groups=RG,
                ins=[st2_in.ap().opt()], outs=[st2_out.ap().opt()],
            )
            st2g = sp.tile([NF, 2], f32, tag="st2g")
            nc.sync.dma_start(out=st2g[:], in_=st2_out[:])
            m2 = sp.tile([NF, 4], f32, tag="m2")
            nc.scalar.mul(m2[:, 0:1], st2g[:, 0:1], 1.0 / N)
            nc.scalar.mul(m2[:, 1:2], st2g[:, 1:2], 1.0 / N)
            nc.scalar.square(m2[:, 2:3], m2[:, 0:1])
            nc.vector.tensor_sub(m2[:, 3:4], m2[:, 1:2], m2[:, 2:3])
            nc.vector.tensor_scalar_add(m2[:, 3:4], m2[:, 3:4], EPS)
            nc.scalar.sqrt(m2[:, 2:3], m2[:, 3:4])
            nc.vector.reciprocal(m2[:, 3:4], m2[:, 2:3])
            sc2 = sp.tile([NF, 2], f32, tag="sc2")
            nc.vector.tensor_mul(sc2[:, 0:1], vecs[:NF, 3:4], m2[:, 3:4])
            nc.vector.tensor_mul(m2[:, 2:3], m2[:, 0:1], sc2[:, 0:1])
            nc.vector.tensor_sub(sc2[:, 1:2], vecs[:NF, 4:5], m2[:, 2:3])

            # ---------------- phase 3: t2 table ----------------
            for s in range(NSUP):
                t2t = sp.tile([NF, ST], bf16, tag="t2t")
                nc.scalar.activation(
                    t2t[:], haff[:, s * ST : (s + 1) * ST], AF.Silu,
                    bias=sc2[:, 1:2], scale=sc2[:, 0:1],
                )
                t2r = rp.tile([128, 4 * NF], bf16, tag="rT")
                for u in range(4):
                    tps = pst.tile([128, ST], bf16, tag="stg")
                    nc.tensor.transpose(
                        tps[:, :NF], t2t[:, u * 128 : (u + 1) * 128], idn_b_sb[:]
                    )
                    nc.scalar.copy(t2r[:, u * NF : (u + 1) * NF], tps[:, :NF])
                store_rows(t2_bounce, s, t2r, NF)
            nc.gpsimd.collective_compute(
                "AllGather", ALU.bypass, replica_groups=RG,
                ins=[t2_bounce.ap().opt()], outs=[t2_table[:N, :].opt()],
            )

            # ---------------- phase 4: conv2 + idconv -> t3 table ----------------
            for s in range(NSUP):
                g2t = gp.tile([128, K * 4 * NF], bf16, tag="gt")
                nc.gpsimd.indirect_dma_start(
                    out=g2t[:].rearrange("p (m e) -> p m e", e=NF),
                    out_offset=None,
                    in_=t2_table[:],
                    in_offset=bass.IndirectOffsetOnAxis(
                        ap=idx_sb[:, s * K * 4 : (s + 1) * K * 4], axis=0
                    ),
                )
                acc2 = pac.tile([NF, ST], f32, tag="acc")
                xbt = sp.tile([NI, ST], bf16, tag="xbt")
                nc.sync.dma_start(out=xbt[:], in_=xT_b[:, s * ST : (s + 1) * ST])
                nc.tensor.matmul(acc2[:], lhsT=Wid_sb[:], rhs=xbt[:],
                                 start=True, stop=False)
                for k in range(K):
                    stg = pst.tile([128, ST], bf16, tag="stg")
                    for u in range(4):
                        cc = (k * 4 + u) * NF
                        nc.tensor.transpose(
                            stg[:, u * 128 : (u + 1) * 128],
                            g2t[:, cc : cc + NF], idn_b_sb[:],
                        )
                    rhs = rp.tile([128, ST], bf16, tag="rhs")
                    if k % 2 == 0:
                        nc.vector.tensor_copy(rhs[:], stg[:])
                    else:
                        nc.scalar.copy(rhs[:], stg[:])
                    nc.tensor.matmul(
                        acc2[:], lhsT=W2_sb[:, k * NF : (k + 1) * NF],
                        rhs=rhs[:], start=False, stop=(k == K - 1),
                    )
                # h_final = acc2 + (b2+bid), cast bf16
                t3t = sp.tile([NF, ST], bf16, tag="t3t")
                nc.scalar.activation(t3t[:], acc2[:], AF.Identity,
                                     bias=vecs[:NF, 5:6])
                t3r = rp.tile([128, 4 * NF], bf16, tag="rT")
                for u in range(4):
                    tps = pst.tile([128, ST], bf16, tag="stg")
                    nc.tensor.transpose(
                        tps[:, :NF], t3t[:, u * 128 : (u + 1) * 128], idn_b_sb[:]
                    )
                    nc.scalar.copy(t3r[:, u * NF : (u + 1) * NF], tps[:, :NF])
                store_rows(t3_bounce, s, t3r, NF)
            nc.gpsimd.collective_compute(
                "AllGather", ALU.bypass, replica_groups=RG,
                ins=[t3_bounce.ap().opt()], outs=[t3_table[:N, :].opt()],
            )

            # ---------------- phase 5: down conv ----------------
            for td in range(NDT):
                gdt = gp.tile([128, KD * NF], bf16, tag="gdt")
                nc.gpsimd.indirect_dma_start(
                    out=gdt[:].rearrange("p (m e) -> p m e", e=NF),
                    out_offset=None,
                    in_=t3_table[:],
                    in_offset=bass.IndirectOffsetOnAxis(
                        ap=idxd_sb[:, td * KD : (td + 1) * KD], axis=0
                    ),
                )
                accd = pac.tile([NF, ST], f32, tag="acc")
                for kd in range(KD):
                    stg = pst.tile([128, ST], bf16, tag="stg")
                    nc.tensor.transpose(
                        stg[:, :DTILE], gdt[:, kd * NF : (kd + 1) * NF],
                        idn_b_sb[:],
                    )
                    rhs = rp.tile([128, ST], bf16, tag="rhs")
                    if kd % 2 == 0:
                        nc.vector.tensor_copy(rhs[:, :DTILE], stg[:, :DTILE])
                    else:
                        nc.scalar.copy(rhs[:, :DTILE], stg[:, :DTILE])
                    nc.tensor.matmul(
                        accd[:, :DTILE], lhsT=Wd_sb[:, kd * NF : (kd + 1) * NF],
                        rhs=rhs[:, :DTILE], start=(kd == 0), stop=(kd == KD - 1),
                    )
                od_f = sp.tile([NF, DTILE], f32, tag="odf")
                nc.vector.tensor_copy(od_f[:], accd[:, :DTILE])
                ops = pst.tile([128, ST], f32, tag="stg")
                nc.tensor.transpose(ops[:, :NF], od_f[:], idn_f_sb[:])
                od_r = sp.tile([128, NF], f32, tag="odr")
                nc.scalar.copy(od_r[:], ops[:, :NF])
                nc.sync.dma_start(
                    out=out_t[td * DTILE : (td + 1) * DTILE, :], in_=od_r[:]
                )

    nc.compile()
    return nc


# ---------------------------------------------------------------------------
# Host-side sharding / input prep
# ---------------------------------------------------------------------------
def prep_inputs(x, t, b_idx, nbr, nbr_down, g1, be1, W1, b1, Wt, bt,
                g2, be2, W2, b2, Wid, bid, Wd):
    import ml_dtypes

    bf16 = ml_dtypes.bfloat16
    f32 = np.float32

    x = np.asarray(x, f32)
    nbr = np.asarray(nbr)
    nbr_fixed = np.where(nbr < 0, DUMMY, nbr).astype(np.int32)  # [K, N]
    b_idx = np.asarray(b_idx).astype(np.int64)
    nbr_down = np.asarray(nbr_down).astype(np.int32)
    W1 = np.asarray(W1, f32)

    # pair-stacked W1 (+ zero pad for odd k), laid out [128, 14*NF]
    W1p = np.zeros((14, 128, NF), dtype=bf16)
    for kp in range(13):
        W1p[kp, :NI] = W1[2 * kp].astype(bf16)
        W1p[kp, NI:] = W1[2 * kp + 1].astype(bf16)
    W1p[13, :NI] = W1[26].astype(bf16)  # second half stays zero
    W1s = np.ascontiguousarray(W1p.transpose(1, 0, 2).reshape(128, 14 * NF))

    bt_rep = np.broadcast_to(np.asarray(bt, f32), (B, 2 * NF)).copy()
    idn = np.eye(128)

    tTd = np.asarray(t, f32).T.reshape(2, 128, B).transpose(1, 0, 2)
    Wtd = np.asarray(Wt, f32).reshape(2, 128, 2 * NF).transpose(1, 0, 2)
    common = dict(
        tT=np.ascontiguousarray(tTd.reshape(128, 2 * B)),
        Wt=np.ascontiguousarray(Wtd.reshape(128, 4 * NF)),
        bt_rep=bt_rep,
        W1s=W1s,
        W2=np.ascontiguousarray(
            np.asarray(W2, f32).transpose(1, 0, 2).reshape(NF, K * NF)
        ).astype(bf16),
        Wd=np.ascontiguousarray(
            np.asarray(Wd, f32).transpose(1, 0, 2).reshape(NF, KD * NF)
        ).astype(bf16),
        Wid=np.ascontiguousarray(np.asarray(Wid, f32).astype(bf16)),
        g1c=np.ascontiguousarray(np.asarray(g1, f32).reshape(NI, 1)),
        be1c=np.ascontiguousarray(np.asarray(be1, f32).reshape(NI, 1)),
        b1c=np.ascontiguousarray(np.asarray(b1, f32).reshape(NF, 1)),
        g2c=np.ascontiguousarray(np.asarray(g2, f32).reshape(NF, 1)),
        be2c=np.ascontiguousarray(np.asarray(be2, f32).reshape(NF, 1)),
        biasfc=np.ascontiguousarray(
            (np.asarray(b2, f32) + np.asarray(bid, f32)).reshape(NF, 1)
        ),
        idn_b=idn.astype(bf16),
        idn_f=idn.astype(f32),
        ones_b=np.ones((1, ST), dtype=bf16),
    )

    in_maps = []
    for c in range(CORES):
        lo, hi = c * SHARD, (c + 1) * SHARD
        xT = np.zeros((NI, SHARD_PAD), dtype=f32)
        xT[:, :SHARD] = x[lo:hi].T

        # conv neighbor indices: [128, NSUP*K*4], col m = s*K*4 + k*4 + u
        nb_pad = np.full((K, SHARD_PAD), DUMMY, dtype=np.int32)
        nb_pad[:, :SHARD] = nbr_fixed[:, lo:hi]
        # [K, NSUP, 4, 128] -> [128, NSUP, K, 4]
        nbv = nb_pad.reshape(K, NSUP, 4, 128).transpose(3, 1, 0, 2)
        idx1 = np.ascontiguousarray(nbv.reshape(128, NSUP * K * 4))

        # one-hot of b_idx, transposed: [B, SHARD_PAD]
        bh = np.zeros((B, SHARD_PAD), dtype=bf16)
        bh[b_idx[lo:hi], np.arange(SHARD)] = 1

        # down-conv indices: [128, NDT*KD], col m = td*KD + kd
        dlo = c * DSHARD
        nd_pad = np.full((KD, DSHARD_PAD), DUMMY, dtype=np.int32)
        nd_pad[:, :DSHARD] = nbr_down[:, dlo : dlo + DSHARD]
        ndv = nd_pad.reshape(KD, NDT, 128).transpose(2, 1, 0)
        idxd = np.ascontiguousarray(ndv.reshape(128, NDT * KD))

        in_maps.append(
            dict(
                common,
                xT_f=xT,
                xT_b=xT.astype(bf16),
                idx1=idx1,
                idxd=idxd,
                b1hotT=bh,
            )
        )
    return in_maps


def kernel(**inputs):
    from concourse.bass_utils import run_bass_kernel_spmd

    if "nc" not in _COMPILED:
        _COMPILED["nc"] = build_program()
    nc = _COMPILED["nc"]

    inputs = {k: np.asarray(v) for k, v in inputs.items()}
    in_maps = prep_inputs(**inputs)
    res = run_bass_kernel_spmd(nc, in_maps, core_ids=list(range(CORES)))
    out = np.concatenate(
        [np.asarray(res.results[c]["out"][:DSHARD]) for c in range(CORES)], axis=0
    )
    return out.astype(np.float32)


if __name__ == "__main__":
    build_program()
    print("program built OK")
